# revision 51
# baseline (speedup 1.0000x reference)
"""Performer (FAVOR+) encoder layer on 8 trn2 NeuronCores.

Sharding: data-parallel over sequence (512 positions per core x 4 batches).
The linear-attention summaries (A = E_k^T v per (batch, head), usum) are
combined in packed AllReduces overlapped with compute.

Algebraic simplifications (validated vs reference, rel-L2 ~4.5e-3):
 - EPS_KERN terms and the global key-feature max are dropped: attn = num/den
   is invariant to any global scaling of kf and per-token scaling of qf, and
   the eps contributions are ~1e-6 relative.
 - The q-side diag (|q|^2 term) is per-token and cancels in num/den, so
   Eq = exp(x @ (Wq . dn . proj^T)) with the projection fused into the
   weights host-side -- the q path is one GEMM plus one Exp.
 - LayerNorm: n = res*A + B with A = bcast(rstd), B = bcast(-mu*rstd);
   rstd = exp(-0.5*ln(var+eps)) on the scalar engine (same activation table
   set as Exp), so the whole chain is 2 scalar + 3 small vector ops.
"""
import os
import numpy as np
import ml_dtypes

B, N, D = 4, 4096, 1024
H, DH = 16, 64
DFF = 4096
M = 64
EPS_LN = 1e-6
NC = 8
NT = N // NC                # 512 positions per core per batch
PAIRS = H // 2              # 8 head-pairs
KT_D = D // 128             # 8
MT_FF = DFF // 128          # 32
TT = NT // 128              # 4
DN = 1.0 / np.sqrt(np.sqrt(DH))
DN2H = DN * DN / 2.0


def _emit(nc, tc, trivial_ln):
    import concourse.mybir as mybir
    from contextlib import ExitStack
    F32 = mybir.dt.float32
    BF16 = mybir.dt.bfloat16
    AF = mybir.ActivationFunctionType
    ALU = mybir.AluOpType

    dram = lambda name, shape, dt, kind: nc.dram_tensor(name, shape, dt, kind=kind).ap()

    x_bf = dram("x_bf", [B, D, NT], BF16, "ExternalInput")
    wqps = dram("wqps", [KT_D, 128, KT_D, 128], BF16, "ExternalInput")
    wks = dram("wks", [KT_D, 128, KT_D, 128], BF16, "ExternalInput")
    wv = dram("wv", [D, D], BF16, "ExternalInput")
    wos = dram("wos", [KT_D, 128, KT_D, 128], BF16, "ExternalInput")
    w1s = dram("w1s", [MT_FF, 128, KT_D, 128], BF16, "ExternalInput")
    w2s = dram("w2s", [KT_D, 128, MT_FF, 128], BF16, "ExternalInput")
    projbd = dram("projbd", [128, 128], BF16, "ExternalInput")
    negselF = dram("negselF", [2, 128], BF16, "ExternalInput")
    sel2 = dram("sel2", [128, 2], BF16, "ExternalInput")
    sel2b = dram("sel2b", [2, 128], F32, "ExternalInput")
    ones1x128 = dram("ones1x128", [1, 128], F32, "ExternalInput")
    mean1 = dram("mean1", [128, 1], F32, "ExternalInput")
    headmask2 = dram("headmask2", [128, 2], F32, "ExternalInput")
    b1c = dram("b1c", [128, MT_FF], F32, "ExternalInput")
    b1p1c = dram("b1p1c", [128, MT_FF], F32, "ExternalInput")
    b2adjc = dram("b2adjc", [128, KT_D], F32, "ExternalInput")
    if not trivial_ln:
        g1c = dram("g1c", [128, KT_D], F32, "ExternalInput")
        be1c = dram("be1c", [128, KT_D], F32, "ExternalInput")
        g2c = dram("g2c", [128, KT_D], F32, "ExternalInput")
        be2c = dram("be2c", [128, KT_D], F32, "ExternalInput")
    out = dram("out", [B, D, NT], F32, "ExternalOutput")

    AC_A = B * PAIRS * 64
    AC_U = B * PAIRS
    AC = AC_A + AC_U               # 2080
    H1 = 16 * 64 + 16              # first-half AR payload (batches 0/1)

    ctx = ExitStack()
    pconst = ctx.enter_context(tc.tile_pool(name="pconst", bufs=1))
    pstream = ctx.enter_context(tc.tile_pool(name="pstream", bufs=3))
    pw2s = ctx.enter_context(tc.tile_pool(name="pw2s", bufs=4))
    pxa = ctx.enter_context(tc.tile_pool(name="pxa", bufs=1))
    pmt = ctx.enter_context(tc.tile_pool(name="pmt", bufs=4))
    psm = ctx.enter_context(tc.tile_pool(name="psm", bufs=2))
    peq = ctx.enter_context(tc.tile_pool(name="peq", bufs=1))
    pbig = ctx.enter_context(tc.tile_pool(name="pbig", bufs=1))
    pone = ctx.enter_context(tc.tile_pool(name="pone", bufs=1))
    pdram = ctx.enter_context(tc.tile_pool(name="pdram", bufs=1, space="DRAM"))
    PP = ctx.enter_context(tc.tile_pool(name="PP", bufs=4, space="PSUM"))
    PD = ctx.enter_context(tc.tile_pool(name="PD", bufs=2, space="PSUM"))
    PS = ctx.enter_context(tc.tile_pool(name="PS", bufs=2, space="PSUM"))

    # ---- first the tensors stage A(b=0) needs, then the rest ----
    xbf0 = pxa.tile([128, KT_D, NT], BF16, tag="xbf", bufs=2)
    nc.sync.dma_start(xbf0[:], x_bf[0].rearrange("(kt p) t -> p kt t", p=128))
    # pre-issue the first two K-weight chunks so the PE starts ~10us earlier
    wk_pre = {}
    for pr in (0, 1):
        t = pstream.tile([128, KT_D, 128], BF16, tag="wmt")
        nc.sync.dma_start(t[:], wks[pr])
        wk_pre[pr] = t
    cAPs = {}

    def load_const(name, ap, shape, dt):
        t = pconst.tile(shape, dt, tag=name)
        nc.sync.dma_start(t[:], ap[:])
        cAPs[name] = t

    load_const("projbd", projbd, [128, 128], BF16)
    load_const("negselF", negselF, [2, 128], BF16)
    load_const("sel2", sel2, [128, 2], BF16)
    # wv shares the big32 slot with hsb: wv is only read in stage A,
    # hsb only from FFN1 onward -- disjoint uses.
    wv_sb = pbig.tile([128, KT_D, D], BF16, tag="big32")
    nc.sync.dma_start(wv_sb[:], wv.rearrange("(kt p) m -> p kt m", p=128))
    for name, ap, shape, dt in (
        ("sel2b", sel2b, [2, 128], F32),
        ("ones1x128", ones1x128, [1, 128], F32),
        ("mean1", mean1, [128, 1], F32), ("headmask2", headmask2, [128, 2], F32),
        ("b1c", b1c, [128, MT_FF], F32), ("b1p1c", b1p1c, [128, MT_FF], F32),
        ("b2adjc", b2adjc, [128, KT_D], F32),
    ):
        load_const(name, ap, shape, dt)
    if not trivial_ln:
        load_const("g1c", g1c, [128, KT_D], F32)
        load_const("be1c", be1c, [128, KT_D], F32)
        load_const("g2c", g2c, [128, KT_D], F32)
        load_const("be2c", be2c, [128, KT_D], F32)
    mean1_bf = pconst.tile([128, 1], BF16, tag="mean1bf")
    sel2b_bf = pconst.tile([2, 128], BF16, tag="sel2bbf")
    ones1x128_bf = pconst.tile([1, 128], BF16, tag="ones1x128bf")
    nc.vector.tensor_copy(mean1_bf[:], cAPs["mean1"][:])
    nc.vector.tensor_copy(sel2b_bf[:], cAPs["sel2b"][:])
    nc.vector.tensor_copy(ones1x128_bf[:], cAPs["ones1x128"][:])
    magicrow = pconst.tile([1, NT], mybir.dt.int32, tag="magicrow")
    nc.vector.memset(magicrow[:], 0x5f3759df)
    c15 = pconst.tile([1, 1], F32, tag="c15")
    nc.vector.memset(c15[:], 1.5)


    arstage = pone.tile([128, AC], F32, tag="arbuf")
    arin1 = pdram.tile([128, H1], F32, tag="arin1")
    arout1 = pdram.tile([128, H1], F32, tag="arout1")
    SEG = 8 * 64 + 8               # per-batch payload for batches 2/3
    arinb = pdram.tile([128, SEG], F32, tag="arinb")
    aroutb = pdram.tile([128, SEG], F32, tag="aroutb")
    arinc = pdram.tile([128, SEG], F32, tag="arinc")
    aroutc = pdram.tile([128, SEG], F32, tag="aroutc")

    def fire_ar(arin, arout, lo, hi):
        nc.sync.dma_start(arin[:], arstage[:, lo:hi])
        if os.environ.get("KERNEL_NOCOLL"):
            nc.sync.dma_start(arout[:], arin[:])
        else:
            nc.gpsimd.collective_compute("AllReduce", ALU.add,
                                         replica_groups=[list(range(NC))],
                                         ins=[arin[:]], outs=[arout[:]])
        nc.sync.dma_start(arstage[:, lo:hi], arout[:])

    # kv summaries live here; memset early (no dependencies)
    kvBall = pone.tile([128, B * PAIRS, 130], BF16, tag="kvBall")
    nc.vector.memset(kvBall[:], 0.0)

    def kv_unpack(off, j0, np_):
        """Unpack an AllReduced arstage segment into kvBall[j0:j0+np_]."""
        jsl = slice(j0, j0 + np_)
        arA0 = arstage[0:64, off:off + np_ * 64].rearrange(
            "p (j c) -> p j c", j=np_)
        nc.vector.tensor_copy(kvBall[0:64, jsl, 0:64], arA0)
        arA1 = arstage[64:128, off:off + np_ * 64].rearrange(
            "p (j c) -> p j c", j=np_)
        nc.scalar.activation(kvBall[64:128, jsl, 64:128], arA1, AF.Copy)
        usl = arstage[:, off + np_ * 64:off + np_ * 64 + np_]
        nc.vector.tensor_scalar(kvBall[:, jsl, 128:129], usl,
                                cAPs["headmask2"][:, 0:1], None, op0=ALU.mult)
        nc.vector.tensor_scalar(kvBall[:, jsl, 129:130], usl,
                                cAPs["headmask2"][:, 1:2], None, op0=ALU.mult)

    # ================= stage A =================
    for b in range(B):
        if b == 0:
            xbf = xbf0
        else:
            xbf = pxa.tile([128, KT_D, NT], BF16, tag="xbf", bufs=2)
            nc.sync.dma_start(xbf[:], x_bf[b].rearrange("(kt p) t -> p kt t", p=128))

        kTs, ksq2s, Eks = {}, {}, {}

        def s1(pr):  # K projection -> token-major k + k^2 (vector trails)
            if b == 0 and pr in wk_pre:
                wkmt = wk_pre[pr]
            else:
                wkmt = pstream.tile([128, KT_D, 128], BF16, tag="wmt")
                nc.sync.dma_start(wkmt[:], wks[pr])
            pk = PP.tile([128, NT], F32, tag="pbig")
            for kt in range(KT_D):
                nc.tensor.matmul(pk[:], wkmt[:, kt, :], xbf[:, kt, :],
                                 start=kt == 0, stop=kt == KT_D - 1)
            kTmt = pmt.tile([128, NT], BF16, tag="mt512", bufs=4)
            nc.vector.tensor_copy(kTmt[:], pk[:])
            ksqmt = pmt.tile([128, NT], BF16, tag="mt512", bufs=4)
            nc.vector.tensor_tensor(ksqmt[:], kTmt[:], kTmt[:], op=ALU.mult)
            kTs[pr] = (kTmt, ksqmt)

        def emit_vtok():
            vt = pxa.tile([128, TT, PAIRS, 129], BF16, tag="vtok", bufs=2)
            nc.vector.memset(vt[:, :, :, 128:129], 1.0)
            for tt in range(TT):
                for nh in range(2):
                    pv = PP.tile([128, 4, 128], F32, tag="pbig")
                    for kt in range(KT_D):
                        nc.tensor.matmul(pv[:], xbf[:, kt, tt * 128:(tt + 1) * 128],
                                         wv_sb[:, kt, nh * 512:(nh + 1) * 512],
                                         start=kt == 0, stop=kt == KT_D - 1)
                    nc.vector.tensor_copy(vt[:, tt, nh * 4:(nh + 1) * 4, 0:128],
                                          pv[:])
            return vt

        def s2(pr):  # squared-norm row + FAVOR features
            kTmt, ksqmt = kTs[pr]
            pks = PS.tile([33, NT], F32, tag="pstat")
            nc.tensor.matmul(pks[0:2, :], cAPs["sel2"][:], ksqmt[:], start=True,
                             stop=True)
            ksq2 = psm.tile([2, NT], BF16, tag="ksq2")
            nc.scalar.activation(ksq2[:], pks[0:2, :], AF.Copy)
            ksq2s[pr] = ksq2

            # One accumulation group for the whole bank: only the FIRST matmul
            # carries start=True; negsels then accumulate onto set bits.
            Ek = psm.tile([128, TT, 128], BF16, tag="Ek")
            pdd = PD.tile([128, NT], F32, tag="pdd")
            sls = [slice(tt * 128, (tt + 1) * 128) for tt in range(TT)]
            for tt in range(TT):
                nc.tensor.matmul(pdd[:, sls[tt]], kTmt[:, sls[tt]],
                                 cAPs["projbd"][:], start=tt == 0, stop=False,
                                 skip_group_check=True)
            for tt in range(TT):
                nc.tensor.matmul(pdd[:, sls[tt]], ksq2[:, sls[tt]],
                                 cAPs["negselF"][:], start=False,
                                 stop=tt == TT - 1, skip_group_check=True)
            nc.scalar.activation(Ek[:], pdd[:], AF.Exp)
            Eks[pr] = Ek

        def s3(pr):  # token-contraction A matmuls + AR staging
            Ek = Eks[pr]
            pA = PD.tile([128, 129], F32, tag="pdd")
            for tt in range(TT):
                nc.tensor.matmul(pA[:], Ek[:, tt, :], vtok[:, tt, pr, :],
                                 start=tt == 0, stop=tt == TT - 1,
                                 skip_group_check=True)
            j = b * PAIRS + pr
            if j < 16:
                acol, ucol = j * 64, 1024 + j
            elif j < 24:
                acol, ucol = H1 + (j - 16) * 64, H1 + 512 + (j - 16)
            else:
                off = H1 + SEG
                acol, ucol = off + (j - 24) * 64, off + 512 + (j - 24)
            nc.vector.tensor_copy(arstage[0:64, acol:acol + 64], pA[0:64, 0:64])
            nc.vector.tensor_copy(arstage[64:128, acol:acol + 64],
                                  pA[64:128, 64:128])
            nc.vector.tensor_copy(arstage[:, ucol:ucol + 1], pA[:, 128:129])

        if b == 0:
            # start the K chain before the (wv-gated) v block so the PE has
            # work as soon as the first small DMAs land.
            s1(0)
            s1(1)
            vtok = emit_vtok()
            s2(0)
            for step in range(2, PAIRS + 2):
                if step < PAIRS:
                    s1(step)
                if step <= PAIRS:
                    s2(step - 1)
                s3(step - 2)
        else:
            vtok = emit_vtok()
            for step in range(PAIRS + 2):
                if step < PAIRS:
                    s1(step)
                if 1 <= step <= PAIRS:
                    s2(step - 1)
                if step >= 2:
                    s3(step - 2)

        if b == 1:
            # batches 0/1 AllReduce rides under stage A(2,3); result lands
            # back in arstage's first-half columns in place.
            fire_ar(arin1, arout1, 0, H1)
        elif b == 2:
            # batch 0/1 summaries have landed by now -- unpack them while
            # stage A(3) runs, then fire batch 2's (small) AllReduce.
            kv_unpack(0, 0, 16)
            fire_ar(arinb, aroutb, H1, H1 + SEG)
        elif b == 3:
            fire_ar(arinc, aroutc, H1 + SEG, AC)

    # ================= q-side features (overlap AR) =================
    # Eq = exp(x @ Wq') with the FAVOR projection fused into Wq' host-side;
    # the per-token diag term cancels in num/den and is dropped.
    Eq_all = {}

    def emit_qside(b):
        # shares the vtok slot: vtok's last stage-A read precedes qside
        qxbf = pxa.tile([128, KT_D, NT], BF16, tag="vtok", bufs=2)
        nc.sync.dma_start(qxbf[:], x_bf[b].rearrange("(kt p) t -> p kt t", p=128))
        Eqs = [None] * PAIRS
        Eq_all[b] = Eqs
        pqs = {}

        def q1(pr):
            wqmt = pstream.tile([128, KT_D, 128], BF16, tag="wmt")
            nc.sync.dma_start(wqmt[:], wqps[pr])
            pq_ = PP.tile([128, NT], F32, tag="pbig")
            for kt in range(KT_D):
                nc.tensor.matmul(pq_[:], wqmt[:, kt, :], qxbf[:, kt, :],
                                 start=kt == 0, stop=kt == KT_D - 1)
            pqs[pr] = pq_

        def q2(pr):
            Eq = peq.tile([128, NT], BF16, tag=f"Eq{b % 3}_{pr}")
            nc.scalar.activation(Eq[:], pqs[pr][:], AF.Exp)
            Eqs[pr] = Eq

        for step in range(PAIRS + 1):
            if step < PAIRS:
                q1(step)
            if step >= 1:
                q2(step - 1)

    kvB_all = {j: kvBall[:, j, :] for j in range(B * PAIRS)}

    # ================= stage B =================
    def attn_pass1(b, attnT, rdens):
        Eqs = Eq_all[b]

        def p1(pr):
            kvB = kvB_all[b * PAIRS + pr]
            pnum = PP.tile([128, NT], F32, tag="pbig")
            nc.tensor.matmul(pnum[:], kvB[:, 0:128], Eqs[pr][:], start=True,
                             stop=True)
            pden = PD.tile([2, NT], F32, tag="pdd")
            nc.tensor.matmul(pden[:], kvB[:, 128:130], Eqs[pr][:], start=True,
                             stop=True)
            nc.scalar.activation(attnT[:, pr, :], pnum[:], AF.Copy)
            rdf = psm.tile([2, NT], F32, tag="lnden", bufs=1)
            nc.vector.reciprocal_approx_fast(rdf[:], pden[:])
            rdb = psm.tile([2, NT], BF16, tag="rden", bufs=2)
            nc.scalar.activation(rdb[:], rdf[:], AF.Copy)
            rdens[pr] = rdb

        for pr in range(PAIRS):
            p1(pr)

    def attn_pass2(b, attnT, rdens):
        def p2(pr):
            prdB = PP.tile([128, NT], F32, tag="pbig")
            nc.tensor.matmul(prdB[:], sel2b_bf[:], rdens[pr][:],
                             start=True, stop=True)
            nc.vector.tensor_tensor(attnT[:, pr, :], attnT[:, pr, :], prdB[:],
                                    op=ALU.mult)

        for pr in range(PAIRS):
            p2(pr)

    def stats_tiles():
        pm = PS.tile([33, NT], F32, tag="pstat")
        return pm

    def stats_step(pm, res_slice, first, last):
        sqt = psm.tile([128, NT], BF16, tag="lnsq")
        nc.vector.tensor_tensor(sqt[:], res_slice, res_slice, op=ALU.mult)
        nc.tensor.matmul(pm[0:1, :], mean1_bf[:], res_slice,
                         start=first, stop=last, skip_group_check=True)
        nc.tensor.matmul(pm[32:33, :], mean1_bf[:], sqt[:],
                         start=first, stop=last, skip_group_check=True)

    def ln_chain(pm):
        I32 = mybir.dt.int32
        mu_sb = psm.tile([1, NT], BF16, tag="lnmu", bufs=2)
        nc.scalar.activation(mu_sb[:], pm[0:1, :], AF.Copy)
        mu2 = psm.tile([1, NT], F32, tag="lnrow")
        nc.scalar.square(mu2[:], mu_sb[:])
        vare = psm.tile([1, NT], F32, tag="lnvare", bufs=1)
        nc.vector.scalar_tensor_tensor(vare[:], pm[32:33, :], float(EPS_LN),
                                       mu2[:], op0=ALU.add, op1=ALU.subtract)
        # rstd via bit-magic Newton on the vector engine (no act-table cost)
        sh = psm.tile([1, NT], I32, tag="lnrow")
        nc.vector.tensor_scalar(sh[:], vare[:].bitcast(I32), 1, None,
                                op0=ALU.arith_shift_right)
        y0 = psm.tile([1, NT], I32, tag="lnya", bufs=1)
        nc.vector.tensor_tensor(y0[:], magicrow[:], sh[:], op=ALU.subtract)
        y = y0[:].bitcast(F32)
        t1 = psm.tile([1, NT], F32, tag="lnrow")
        nc.vector.tensor_tensor(t1[:], y, y, op=ALU.mult)
        t2 = psm.tile([1, NT], F32, tag="lnrow")
        nc.vector.tensor_tensor(t2[:], t1[:], vare[:], op=ALU.mult)
        t3 = psm.tile([1, NT], F32, tag="lnrow")
        nc.scalar.activation(t3[:], t2[:], AF.Identity, bias=c15[:], scale=-0.5)
        rstd = psm.tile([1, NT], BF16, tag="lnrstd", bufs=2)
        nc.vector.tensor_tensor(rstd[:], y, t3[:], op=ALU.mult)
        nmr = psm.tile([1, NT], BF16, tag="lnnmr", bufs=2)
        nc.vector.scalar_tensor_tensor(nmr[:], mu_sb[:], -1.0, rstd[:],
                                       op0=ALU.mult, op1=ALU.mult)
        return rstd, nmr

    def ln_ab(rstd, nmr):
        pa = PD.tile([128, NT], F32, tag="pdd")
        nc.tensor.matmul(pa[:], ones1x128_bf[:], rstd[:], start=True, stop=True)
        a_bf = psm.tile([128, NT], BF16, tag="lnA", bufs=2)
        nc.scalar.activation(a_bf[:], pa[:], AF.Copy)
        pb = PD.tile([128, NT], F32, tag="pdd")
        nc.tensor.matmul(pb[:], ones1x128_bf[:], nmr[:], start=True, stop=True)
        b_bf = psm.tile([128, NT], BF16, tag="lnB", bufs=2)
        nc.scalar.activation(b_bf[:], pb[:], AF.Copy)
        return a_bf, b_bf

    def ln_apply(a_bf, b_bf, res, gc, bc, odt, dma_out=None):
        o = None
        if odt is not None:
            o = pbig.tile([128, KT_D, NT], odt, tag="bf8")
        for kt in range(KT_D):
            t1 = psm.tile([128, NT], BF16, tag="lnt1")
            nc.vector.tensor_tensor(t1[:], res[:, kt, :], a_bf[:], op=ALU.mult)
            if trivial_ln:
                if o is not None:
                    nc.vector.tensor_tensor(o[:, kt, :], t1[:], b_bf[:],
                                            op=ALU.add)
                else:
                    ot = psm.tile([128, NT], F32, tag="lnot")
                    nc.vector.tensor_tensor(ot[:], t1[:], b_bf[:], op=ALU.add)
                    nc.sync.dma_start(
                        dma_out[0][dma_out[1], kt * 128:(kt + 1) * 128, :],
                        ot[:])
            else:
                nrm = psm.tile([128, NT], BF16, tag="lnnrm")
                nc.vector.tensor_tensor(nrm[:], t1[:], b_bf[:], op=ALU.add)
                if o is not None:
                    nc.vector.tensor_scalar(o[:, kt, :], nrm[:],
                                            gc[:, kt:kt + 1], bc[:, kt:kt + 1],
                                            op0=ALU.mult, op1=ALU.add)
                else:
                    ot = psm.tile([128, NT], F32, tag="lnot")
                    nc.vector.tensor_scalar(ot[:], nrm[:], gc[:, kt:kt + 1],
                                            bc[:, kt:kt + 1],
                                            op0=ALU.mult, op1=ALU.add)
                    nc.sync.dma_start(
                        dma_out[0][dma_out[1], kt * 128:(kt + 1) * 128, :],
                        ot[:])
        return o

    def wo_res1(b, attnT):
        res1 = pbig.tile([128, KT_D, NT], BF16, tag="resX", bufs=2)
        pm = stats_tiles()
        for mt in range(KT_D):
            womt = pstream.tile([128, KT_D, 128], BF16, tag="wmt")
            nc.sync.dma_start(womt[:], wos[mt])
            po = PP.tile([128, NT], F32, tag="pbig")
            for kt in range(KT_D):
                nc.tensor.matmul(po[:], womt[:, kt, :], attnT[:, kt, :],
                                 start=kt == 0, stop=kt == KT_D - 1)
            xf = psm.tile([128, NT], BF16, tag="xf")
            nc.sync.dma_start(xf[:], x_bf[b, mt * 128:(mt + 1) * 128, :])
            nc.vector.tensor_tensor(res1[:, mt, :], xf[:], po[:], op=ALU.add)
        # stats as one block AFTER the Wo stream: by the time the PE FIFO
        # reaches them the res1 adds (queued behind the LN applies on the
        # vector engine) have drained, so the stats matmuls never stall.
        for mt in range(KT_D):
            stats_step(pm, res1[:, mt, :], mt == 0, mt == KT_D - 1)
        return res1, pm

    def ffn(b, out1, defer_n=1):
        hsb = pbig.tile([128, MT_FF, NT], BF16, tag="big32")
        res2 = pbig.tile([128, KT_D, NT], BF16, tag="resX", bufs=2)
        pm = stats_tiles()

        g0 = {}

        def ffn2_head(mt, kt_end):
            w2a = pw2s.tile([128, MT_FF // 2, 128], BF16, tag="w2mt")
            nc.sync.dma_start(w2a[:], w2s[mt, :, 0:MT_FF // 2])
            w2b = pw2s.tile([128, MT_FF // 2, 128], BF16, tag="w2mt")
            nc.sync.dma_start(w2b[:], w2s[mt, :, MT_FF // 2:MT_FF])
            pf = PP.tile([128, NT], F32, tag="pbig")
            for kt in range(kt_end):
                w2h = w2a if kt < MT_FF // 2 else w2b
                nc.tensor.matmul(pf[:], w2h[:, kt % (MT_FF // 2), :],
                                 hsb[:, kt, :],
                                 start=kt == 0, stop=False,
                                 skip_group_check=True)
            return pf, w2a, w2b

        def ffn2_tail(mt, pf, w2a, w2b, kt_start):
            for kt in range(kt_start, MT_FF):
                w2h = w2a if kt < MT_FF // 2 else w2b
                nc.tensor.matmul(pf[:], w2h[:, kt % (MT_FF // 2), :],
                                 hsb[:, kt, :],
                                 start=kt == 0, stop=kt == MT_FF - 1,
                                 skip_group_check=True)
            nc.vector.scalar_tensor_tensor(res2[:, mt, :], pf[:],
                                           cAPs["b2adjc"][:, mt:mt + 1],
                                           out1[:, mt, :], op0=ALU.add,
                                           op1=ALU.add)

        def ffn2_group(mt):
            pf, w2a, w2b = ffn2_head(mt, 0)
            ffn2_tail(mt, pf, w2a, w2b, 0)

        for mt in range(MT_FF):
            w1mt = pstream.tile([128, KT_D, 128], BF16, tag="wmt")
            nc.sync.dma_start(w1mt[:], w1s[mt])
            # 6-deep pz rotation: 4 PP banks + the 2 PD banks.
            if mt % 3 != 2:
                pz = PP.tile([128, NT], F32, tag="pbig", name="pz")
            else:
                pz = PD.tile([128, NT], F32, tag="pdd", name="pz")
            for kt in range(KT_D):
                nc.tensor.matmul(pz[:], w1mt[:, kt, :], out1[:, kt, :],
                                 start=kt == 0, stop=kt == KT_D - 1)
            # elu(z')+1 = max(z'+1, min(exp(z'), 1)): one scalar Exp, with the
            # min on the vector engine in 16-bit 2x mode.
            eraw = pmt.tile([128, NT], BF16, tag="t512bf", bufs=3)
            nc.scalar.activation(eraw[:], pz[:], AF.Exp,
                                 bias=cAPs["b1c"][:, mt:mt + 1])
            emin = pmt.tile([128, NT], BF16, tag="t512bf", bufs=3)
            nc.vector.tensor_scalar(emin[:], eraw[:], 1.0, None, op0=ALU.min)
            nc.vector.scalar_tensor_tensor(hsb[:, mt, :], pz[:],
                                           cAPs["b1p1c"][:, mt:mt + 1], emin[:],
                                           op0=ALU.add, op1=ALU.max)
            # FFN2's first accumulation group starts before FFN1's last
            # tiles (only the already-written hsb kts), so the seam never
            # waits on the trailing hsb chain.
            if mt == MT_FF - 3:
                g0['h'] = ffn2_head(0, MT_FF - 2)

        ffn2_tail(0, *g0['h'], MT_FF - 2)
        n_inline = KT_D - defer_n
        for mt in range(1, n_inline):
            ffn2_group(mt)
        # stats as one block AFTER the inline groups: each group's stats no
        # longer stall the PE FIFO on that group's trailing res2 vector op.
        for mt in range(n_inline):
            stats_step(pm, res2[:, mt, :], mt == 0,
                       defer_n == 0 and mt == KT_D - 1)
        if defer_n == 0:
            return res2, pm, None

        # the last FFN2 group(s) are deferred into the next batch's window,
        # where their (ready) matmuls cover the LN chain latencies.
        def defer():
            for mt in range(n_inline, KT_D):
                ffn2_group(mt)
            for mt in range(n_inline, KT_D):
                stats_step(pm, res2[:, mt, :], False, mt == KT_D - 1)
        return res2, pm, defer

    # pre-loop: batch 0/1 kv summaries landed during stage A, so attention
    # for batch 0 interleaves with the q-side GEMMs -- the PE stream stays
    # dense while the remaining AllReduces complete in the background.
    kv_unpack(H1, 16, 8)
    emit_qside(0)
    attnT0 = pbig.tile([128, KT_D, NT], BF16, tag="attnT")
    rd0 = {}
    attn_pass1(0, attnT0, rd0)
    emit_qside(1)
    attn_pass2(0, attnT0, rd0)
    emit_qside(2)
    res1, pm1t = wo_res1(0, attnT0)

    gN = (lambda k: cAPs[k]) if not trivial_ln else (lambda k: None)
    res2p = pm2p = defer2 = None
    for b in range(B):
        # LN2 of the previous batch is handled HERE so this batch's (ready)
        # attention/Wo matmuls cover both LN chains' serial latency.
        if defer2 is not None:
            defer2()
        rn2 = ln_chain(pm2p) if pm2p is not None else None
        rstd1, nmr1 = ln_chain(pm1t)
        attnT = rdn = None
        if b + 1 < B:
            attnT = pbig.tile([128, KT_D, NT], BF16, tag="attnT")
            rdn = {}
            attn_pass1(b + 1, attnT, rdn)
        if rn2 is not None:
            a2, b2t = ln_ab(*rn2)
        a1, b1t = ln_ab(rstd1, nmr1)
        if attnT is not None:
            attn_pass2(b + 1, attnT, rdn)
        out1 = ln_apply(a1, b1t, res1, gN("g1c"), gN("be1c"), BF16)
        if rn2 is not None:
            ln_apply(a2, b2t, res2p, gN("g2c"), gN("be2c"), None,
                     dma_out=(out, b - 1))
        if b == 0:
            emit_qside(3)
            kv_unpack(H1 + SEG, 24, 8)
        if b + 1 < B:
            res1, pm1t = wo_res1(b + 1, attnT)
        # the no-filler final iteration gets TWO deferred groups of PE cover
        res2p, pm2p, defer2 = ffn(b, out1,
                                  defer_n=0 if b == B - 1 else
                                  (2 if b == B - 2 else 1))

    rn2 = ln_chain(pm2p)
    a2, b2t = ln_ab(*rn2)
    ln_apply(a2, b2t, res2p, gN("g2c"), gN("be2c"), None, dma_out=(out, B - 1))

    ctx.close()


_CACHE = {}


def _build(trivial_ln):
    import concourse.tile as tile
    from concourse import bacc
    nc = bacc.Bacc("TRN2", target_bir_lowering=False, debug=False, num_devices=NC)
    with tile.TileContext(nc) as tc:
        _emit(nc, tc, trivial_ln)
    nc.compile()
    return nc


def _host_inputs(x, Wq, Wk, Wv, Wo, proj, W1, b1, W2, b2,
                 ln1_g, ln1_b, ln2_g, ln2_b, trivial_ln):
    bf = ml_dtypes.bfloat16
    f32 = np.float32
    d = {}

    def chunked(w):  # [D, X] -> [X/128 mt, 128 p, D/128 kt, 128]
        Dk, X = w.shape
        r = w.reshape(Dk // 128, 128, X // 128, 128)
        return np.ascontiguousarray(r.transpose(2, 1, 0, 3)).astype(bf)

    # fused q-side weights: Wq' = einsum(Wq, proj*dn) -> [D, H*M]
    Wqp = np.einsum('dhk,mk->dhm', Wq.astype(np.float64),
                    (proj.astype(np.float64) * DN)).reshape(D, H * M)
    d["wqps"] = chunked(Wqp.astype(f32))
    d["wks"] = chunked(Wk.reshape(D, D))
    d["wv"] = np.ascontiguousarray(Wv.reshape(D, D)).astype(bf)
    d["wos"] = chunked(Wo.reshape(D, D))
    d["w1s"] = chunked(W1)
    d["w2s"] = chunked(W2)

    projT_s = (proj * DN).T.astype(f32)
    pbd = np.zeros((128, 128), f32)
    pbd[0:64, 0:64] = projT_s
    pbd[64:128, 64:128] = projT_s
    d["projbd"] = pbd.astype(bf)
    nsF = np.zeros((2, 128), f32)
    nsF[0, 0:64] = -DN2H
    nsF[1, 64:128] = -DN2H
    d["negselF"] = nsF.astype(bf)
    s2 = np.zeros((128, 2), f32)
    s2[0:64, 0] = 1.0
    s2[64:128, 1] = 1.0
    d["sel2"] = s2.astype(bf)
    s2b = np.zeros((2, 128), f32)
    s2b[0, 0:64] = 1.0
    s2b[1, 64:128] = 1.0
    d["sel2b"] = s2b
    d["ones1x128"] = np.ones((1, 128), f32)
    d["mean1"] = np.full((128, 1), 1.0 / D, f32)
    hm2 = np.zeros((128, 2), f32)
    hm2[0:64, 0] = 1.0
    hm2[64:128, 1] = 1.0
    d["headmask2"] = hm2

    d["b1c"] = np.ascontiguousarray(b1.reshape(MT_FF, 128).T).astype(f32)
    d["b1p1c"] = np.ascontiguousarray((b1 + 1.0).reshape(MT_FF, 128).T).astype(f32)
    b2adj = b2.astype(np.float64) - W2.astype(np.float64).sum(axis=0)
    d["b2adjc"] = np.ascontiguousarray(b2adj.reshape(KT_D, 128).T).astype(f32)
    if not trivial_ln:
        d["g1c"] = np.ascontiguousarray(ln1_g.reshape(KT_D, 128).T).astype(f32)
        d["be1c"] = np.ascontiguousarray(ln1_b.reshape(KT_D, 128).T).astype(f32)
        d["g2c"] = np.ascontiguousarray(ln2_g.reshape(KT_D, 128).T).astype(f32)
        d["be2c"] = np.ascontiguousarray(ln2_b.reshape(KT_D, 128).T).astype(f32)
    return d


def kernel(x, Wq, Wk, Wv, Wo, proj, W1, b1, W2, b2, ln1_g, ln1_b, ln2_g, ln2_b):
    from concourse import bass_utils

    x = np.asarray(x, np.float32)
    ln1_g = np.asarray(ln1_g); ln1_b = np.asarray(ln1_b)
    ln2_g = np.asarray(ln2_g); ln2_b = np.asarray(ln2_b)
    trivial_ln = (np.allclose(ln1_g, 1.0) and np.allclose(ln2_g, 1.0)
                  and np.allclose(ln1_b, 0.0) and np.allclose(ln2_b, 0.0))
    shared = _host_inputs(x, np.asarray(Wq), np.asarray(Wk), np.asarray(Wv),
                          np.asarray(Wo), np.asarray(proj), np.asarray(W1),
                          np.asarray(b1), np.asarray(W2), np.asarray(b2),
                          ln1_g, ln1_b, ln2_g, ln2_b, trivial_ln)

    key = ("nc", trivial_ln)
    if key not in _CACHE:
        _CACHE[key] = _build(trivial_ln)
    nc = _CACHE[key]

    in_maps = []
    for c in range(NC):
        xs = x[:, c * NT:(c + 1) * NT, :]
        xT = np.ascontiguousarray(xs.transpose(0, 2, 1))
        m = dict(shared)
        m["x_bf"] = xT.astype(ml_dtypes.bfloat16)
        in_maps.append(m)

    trace = bool(int(os.environ.get("KERNEL_TRACE", "0")))
    res = bass_utils.run_bass_kernel_spmd(nc, in_maps, core_ids=list(range(NC)),
                                          trace=trace)
    if trace and res.exec_time_ns is not None:
        print(f"HW exec time: {res.exec_time_ns} ns")
        if res.instructions_and_trace is not None:
            print("trace:", res.instructions_and_trace[1])

    outp = np.empty((B, N, D), np.float32)
    for c in range(NC):
        oT = res.results[c]["out"]
        outp[:, c * NT:(c + 1) * NT, :] = oT.transpose(0, 2, 1)
    return outp


# revision 52
# speedup vs baseline: 1.0255x; 1.0255x over previous
"""Performer (FAVOR+) encoder layer on 8 trn2 NeuronCores.

Sharding: data-parallel over sequence (512 positions per core x 4 batches).
The linear-attention summaries (A = E_k^T v per (batch, head), usum) are
combined in packed AllReduces overlapped with compute.

Algebraic simplifications (validated vs reference, rel-L2 ~4.5e-3):
 - EPS_KERN terms and the global key-feature max are dropped: attn = num/den
   is invariant to any global scaling of kf and per-token scaling of qf, and
   the eps contributions are ~1e-6 relative.
 - The q-side diag (|q|^2 term) is per-token and cancels in num/den, so
   Eq = exp(x @ (Wq . dn . proj^T)) with the projection fused into the
   weights host-side -- the q path is one GEMM plus one Exp.
 - LayerNorm: n = res*A + B with A = bcast(rstd), B = bcast(-mu*rstd);
   rstd = exp(-0.5*ln(var+eps)) on the scalar engine (same activation table
   set as Exp), so the whole chain is 2 scalar + 3 small vector ops.
"""
import os
import numpy as np
import ml_dtypes

B, N, D = 4, 4096, 1024
H, DH = 16, 64
DFF = 4096
M = 64
EPS_LN = 1e-6
NC = 8
NT = N // NC                # 512 positions per core per batch
PAIRS = H // 2              # 8 head-pairs
KT_D = D // 128             # 8
MT_FF = DFF // 128          # 32
TT = NT // 128              # 4
DN = 1.0 / np.sqrt(np.sqrt(DH))
DN2H = DN * DN / 2.0


def _emit(nc, tc, trivial_ln):
    import concourse.mybir as mybir
    from contextlib import ExitStack
    F32 = mybir.dt.float32
    BF16 = mybir.dt.bfloat16
    AF = mybir.ActivationFunctionType
    ALU = mybir.AluOpType

    dram = lambda name, shape, dt, kind: nc.dram_tensor(name, shape, dt, kind=kind).ap()

    x_bf = dram("x_bf", [B, D, NT], BF16, "ExternalInput")
    wqps = dram("wqps", [KT_D, 128, KT_D, 128], BF16, "ExternalInput")
    wks = dram("wks", [KT_D, 128, KT_D, 128], BF16, "ExternalInput")
    wv = dram("wv", [D, D], BF16, "ExternalInput")
    wos = dram("wos", [KT_D, 128, KT_D, 128], BF16, "ExternalInput")
    w1s = dram("w1s", [MT_FF, 128, KT_D, 128], BF16, "ExternalInput")
    w2s = dram("w2s", [KT_D, 128, MT_FF, 128], BF16, "ExternalInput")
    projbd = dram("projbd", [128, 128], BF16, "ExternalInput")
    negselF = dram("negselF", [2, 128], BF16, "ExternalInput")
    sel2 = dram("sel2", [128, 2], BF16, "ExternalInput")
    sel2b = dram("sel2b", [2, 128], F32, "ExternalInput")
    ones1x128 = dram("ones1x128", [1, 128], F32, "ExternalInput")
    mean1 = dram("mean1", [128, 1], F32, "ExternalInput")
    headmask2 = dram("headmask2", [128, 2], F32, "ExternalInput")
    b1c = dram("b1c", [128, MT_FF], F32, "ExternalInput")
    b1p1c = dram("b1p1c", [128, MT_FF], F32, "ExternalInput")
    b2adjc = dram("b2adjc", [128, KT_D], F32, "ExternalInput")
    if not trivial_ln:
        g1c = dram("g1c", [128, KT_D], F32, "ExternalInput")
        be1c = dram("be1c", [128, KT_D], F32, "ExternalInput")
        g2c = dram("g2c", [128, KT_D], F32, "ExternalInput")
        be2c = dram("be2c", [128, KT_D], F32, "ExternalInput")
    out = dram("out", [B, D, NT], F32, "ExternalOutput")

    AC_A = B * PAIRS * 64
    AC_U = B * PAIRS
    AC = AC_A + AC_U               # 2080
    H1 = 16 * 64 + 16              # first-half AR payload (batches 0/1)

    ctx = ExitStack()
    pconst = ctx.enter_context(tc.tile_pool(name="pconst", bufs=1))
    pstream = ctx.enter_context(tc.tile_pool(name="pstream", bufs=3))
    pw2s = ctx.enter_context(tc.tile_pool(name="pw2s", bufs=3))
    pxa = ctx.enter_context(tc.tile_pool(name="pxa", bufs=1))
    pmt = ctx.enter_context(tc.tile_pool(name="pmt", bufs=4))
    psm = ctx.enter_context(tc.tile_pool(name="psm", bufs=2))
    peq = ctx.enter_context(tc.tile_pool(name="peq", bufs=1))
    pbig = ctx.enter_context(tc.tile_pool(name="pbig", bufs=1))
    pone = ctx.enter_context(tc.tile_pool(name="pone", bufs=1))
    pdram = ctx.enter_context(tc.tile_pool(name="pdram", bufs=1, space="DRAM"))
    PP = ctx.enter_context(tc.tile_pool(name="PP", bufs=4, space="PSUM"))
    PD = ctx.enter_context(tc.tile_pool(name="PD", bufs=2, space="PSUM"))
    PS = ctx.enter_context(tc.tile_pool(name="PS", bufs=2, space="PSUM"))

    # ---- first the tensors stage A(b=0) needs, then the rest ----
    xbf0 = pxa.tile([128, KT_D, NT], BF16, tag="xbf", bufs=2)
    nc.sync.dma_start(xbf0[:], x_bf[0].rearrange("(kt p) t -> p kt t", p=128))
    # pre-issue the first two K-weight chunks so the PE starts ~10us earlier
    wk_pre = {}
    for pr in (0, 1):
        t = pstream.tile([128, KT_D, 128], BF16, tag="wmt")
        nc.sync.dma_start(t[:], wks[pr])
        wk_pre[pr] = t
    cAPs = {}

    def load_const(name, ap, shape, dt):
        t = pconst.tile(shape, dt, tag=name)
        nc.sync.dma_start(t[:], ap[:])
        cAPs[name] = t

    load_const("projbd", projbd, [128, 128], BF16)
    load_const("negselF", negselF, [2, 128], BF16)
    load_const("sel2", sel2, [128, 2], BF16)
    # wv shares the big32 slot with hsb: wv is only read in stage A,
    # hsb only from FFN1 onward -- disjoint uses.
    wv_sb = pbig.tile([128, KT_D, D], BF16, tag="big32")
    nc.sync.dma_start(wv_sb[:], wv.rearrange("(kt p) m -> p kt m", p=128))
    for name, ap, shape, dt in (
        ("sel2b", sel2b, [2, 128], F32),
        ("ones1x128", ones1x128, [1, 128], F32),
        ("mean1", mean1, [128, 1], F32), ("headmask2", headmask2, [128, 2], F32),
        ("b1c", b1c, [128, MT_FF], F32), ("b1p1c", b1p1c, [128, MT_FF], F32),
        ("b2adjc", b2adjc, [128, KT_D], F32),
    ):
        load_const(name, ap, shape, dt)
    if not trivial_ln:
        load_const("g1c", g1c, [128, KT_D], F32)
        load_const("be1c", be1c, [128, KT_D], F32)
        load_const("g2c", g2c, [128, KT_D], F32)
        load_const("be2c", be2c, [128, KT_D], F32)
    mean1_bf = pconst.tile([128, 1], BF16, tag="mean1bf")
    sel2b_bf = pconst.tile([2, 128], BF16, tag="sel2bbf")
    ones1x128_bf = pconst.tile([1, 128], BF16, tag="ones1x128bf")
    nc.vector.tensor_copy(mean1_bf[:], cAPs["mean1"][:])
    nc.vector.tensor_copy(sel2b_bf[:], cAPs["sel2b"][:])
    nc.vector.tensor_copy(ones1x128_bf[:], cAPs["ones1x128"][:])
    magicrow = pconst.tile([1, NT], mybir.dt.int32, tag="magicrow")
    nc.vector.memset(magicrow[:], 0x5f3759df)
    c15 = pconst.tile([1, 1], F32, tag="c15")
    nc.vector.memset(c15[:], 1.5)


    arstage = pone.tile([128, AC], F32, tag="arbuf")
    arin1 = pdram.tile([128, H1], F32, tag="arin1")
    arout1 = pdram.tile([128, H1], F32, tag="arout1")
    SEG = 8 * 64 + 8               # per-batch payload for batches 2/3
    arinb = pdram.tile([128, SEG], F32, tag="arinb")
    aroutb = pdram.tile([128, SEG], F32, tag="aroutb")
    arinc = pdram.tile([128, SEG], F32, tag="arinc")
    aroutc = pdram.tile([128, SEG], F32, tag="aroutc")

    def fire_ar(arin, arout, lo, hi):
        nc.sync.dma_start(arin[:], arstage[:, lo:hi])
        if os.environ.get("KERNEL_NOCOLL"):
            nc.sync.dma_start(arout[:], arin[:])
        else:
            nc.gpsimd.collective_compute("AllReduce", ALU.add,
                                         replica_groups=[list(range(NC))],
                                         ins=[arin[:]], outs=[arout[:]])
        nc.sync.dma_start(arstage[:, lo:hi], arout[:])

    # kv summaries live here; memset early (no dependencies)
    kvBall = pone.tile([128, B * PAIRS, 130], BF16, tag="kvBall")
    nc.vector.memset(kvBall[:], 0.0)

    def kv_unpack(off, j0, np_):
        """Unpack an AllReduced arstage segment into kvBall[j0:j0+np_]."""
        jsl = slice(j0, j0 + np_)
        arA0 = arstage[0:64, off:off + np_ * 64].rearrange(
            "p (j c) -> p j c", j=np_)
        nc.vector.tensor_copy(kvBall[0:64, jsl, 0:64], arA0)
        arA1 = arstage[64:128, off:off + np_ * 64].rearrange(
            "p (j c) -> p j c", j=np_)
        nc.scalar.activation(kvBall[64:128, jsl, 64:128], arA1, AF.Copy)
        usl = arstage[:, off + np_ * 64:off + np_ * 64 + np_]
        nc.vector.tensor_scalar(kvBall[:, jsl, 128:129], usl,
                                cAPs["headmask2"][:, 0:1], None, op0=ALU.mult)
        nc.vector.tensor_scalar(kvBall[:, jsl, 129:130], usl,
                                cAPs["headmask2"][:, 1:2], None, op0=ALU.mult)

    # ================= stage A =================
    for b in range(B):
        if b == 0:
            xbf = xbf0
        else:
            xbf = pxa.tile([128, KT_D, NT], BF16, tag="xbf", bufs=2)
            nc.sync.dma_start(xbf[:], x_bf[b].rearrange("(kt p) t -> p kt t", p=128))

        kTs, ksq2s, Eks = {}, {}, {}

        def s1(pr):  # K projection -> token-major k + k^2 (vector trails)
            if b == 0 and pr in wk_pre:
                wkmt = wk_pre[pr]
            else:
                wkmt = pstream.tile([128, KT_D, 128], BF16, tag="wmt")
                nc.sync.dma_start(wkmt[:], wks[pr])
            pk = PP.tile([128, NT], F32, tag="pbig")
            for kt in range(KT_D):
                nc.tensor.matmul(pk[:], wkmt[:, kt, :], xbf[:, kt, :],
                                 start=kt == 0, stop=kt == KT_D - 1)
            kTmt = pmt.tile([128, NT], BF16, tag="mt512", bufs=6)
            nc.vector.tensor_copy(kTmt[:], pk[:])
            ksqmt = pmt.tile([128, NT], BF16, tag="mt512", bufs=6)
            nc.vector.tensor_tensor(ksqmt[:], kTmt[:], kTmt[:], op=ALU.mult)
            kTs[pr] = (kTmt, ksqmt)

        def emit_vtok():
            vt = pxa.tile([128, TT, PAIRS, 129], BF16, tag="vtok", bufs=2)
            nc.vector.memset(vt[:, :, :, 128:129], 1.0)
            for tt in range(TT):
                for nh in range(2):
                    pv = PP.tile([128, 4, 128], F32, tag="pbig")
                    for kt in range(KT_D):
                        nc.tensor.matmul(pv[:], xbf[:, kt, tt * 128:(tt + 1) * 128],
                                         wv_sb[:, kt, nh * 512:(nh + 1) * 512],
                                         start=kt == 0, stop=kt == KT_D - 1)
                    nc.vector.tensor_copy(vt[:, tt, nh * 4:(nh + 1) * 4, 0:128],
                                          pv[:])
            return vt

        def s2(pr):  # squared-norm row + FAVOR features
            kTmt, ksqmt = kTs[pr]
            pks = PS.tile([33, NT], F32, tag="pstat")
            nc.tensor.matmul(pks[0:2, :], cAPs["sel2"][:], ksqmt[:], start=True,
                             stop=True)
            ksq2 = psm.tile([2, NT], BF16, tag="ksq2")
            nc.scalar.activation(ksq2[:], pks[0:2, :], AF.Copy)
            ksq2s[pr] = ksq2

            # One accumulation group for the whole bank: only the FIRST matmul
            # carries start=True; negsels then accumulate onto set bits.
            Ek = psm.tile([128, TT, 128], BF16, tag="Ek")
            pdd = PD.tile([128, NT], F32, tag="pdd")
            sls = [slice(tt * 128, (tt + 1) * 128) for tt in range(TT)]
            for tt in range(TT):
                nc.tensor.matmul(pdd[:, sls[tt]], kTmt[:, sls[tt]],
                                 cAPs["projbd"][:], start=tt == 0, stop=False,
                                 skip_group_check=True)
            for tt in range(TT):
                nc.tensor.matmul(pdd[:, sls[tt]], ksq2[:, sls[tt]],
                                 cAPs["negselF"][:], start=False,
                                 stop=tt == TT - 1, skip_group_check=True)
            nc.scalar.activation(Ek[:], pdd[:], AF.Exp)
            Eks[pr] = Ek

        def s3(pr):  # token-contraction A matmuls + AR staging
            Ek = Eks[pr]
            pA = PD.tile([128, 129], F32, tag="pdd")
            for tt in range(TT):
                nc.tensor.matmul(pA[:], Ek[:, tt, :], vtok[:, tt, pr, :],
                                 start=tt == 0, stop=tt == TT - 1,
                                 skip_group_check=True)
            j = b * PAIRS + pr
            if j < 16:
                acol, ucol = j * 64, 1024 + j
            elif j < 24:
                acol, ucol = H1 + (j - 16) * 64, H1 + 512 + (j - 16)
            else:
                off = H1 + SEG
                acol, ucol = off + (j - 24) * 64, off + 512 + (j - 24)
            nc.vector.tensor_copy(arstage[0:64, acol:acol + 64], pA[0:64, 0:64])
            nc.vector.tensor_copy(arstage[64:128, acol:acol + 64],
                                  pA[64:128, 64:128])
            nc.vector.tensor_copy(arstage[:, ucol:ucol + 1], pA[:, 128:129])

        if b == 0:
            # start the K chain before the (wv-gated) v block so the PE has
            # work as soon as the first small DMAs land.
            s1(0)
            s1(1)
            vtok = emit_vtok()
            s2(0)
            for step in range(2, PAIRS + 2):
                if step < PAIRS:
                    s1(step)
                if step <= PAIRS:
                    s2(step - 1)
                s3(step - 2)
        else:
            vtok = emit_vtok()
            for step in range(PAIRS + 2):
                if step < PAIRS:
                    s1(step)
                if 1 <= step <= PAIRS:
                    s2(step - 1)
                if step >= 2:
                    s3(step - 2)

        if b == 1:
            # batches 0/1 AllReduce rides under stage A(2,3); result lands
            # back in arstage's first-half columns in place.
            fire_ar(arin1, arout1, 0, H1)
        elif b == 2:
            # batch 0/1 summaries have landed by now -- unpack them while
            # stage A(3) runs, then fire batch 2's (small) AllReduce.
            kv_unpack(0, 0, 16)
            fire_ar(arinb, aroutb, H1, H1 + SEG)
        elif b == 3:
            fire_ar(arinc, aroutc, H1 + SEG, AC)

    # ================= q-side features (overlap AR) =================
    # Eq = exp(x @ Wq') with the FAVOR projection fused into Wq' host-side;
    # the per-token diag term cancels in num/den and is dropped.
    Eq_all = {}

    def emit_qside(b):
        # shares the vtok slot: vtok's last stage-A read precedes qside
        qxbf = pxa.tile([128, KT_D, NT], BF16, tag="vtok", bufs=2)
        nc.sync.dma_start(qxbf[:], x_bf[b].rearrange("(kt p) t -> p kt t", p=128))
        Eqs = [None] * PAIRS
        Eq_all[b] = Eqs
        pqs = {}

        def q1(pr):
            wqmt = pstream.tile([128, KT_D, 128], BF16, tag="wmt")
            nc.sync.dma_start(wqmt[:], wqps[pr])
            pq_ = PP.tile([128, NT], F32, tag="pbig")
            for kt in range(KT_D):
                nc.tensor.matmul(pq_[:], wqmt[:, kt, :], qxbf[:, kt, :],
                                 start=kt == 0, stop=kt == KT_D - 1)
            pqs[pr] = pq_

        def q2(pr):
            Eq = peq.tile([128, NT], BF16, tag=f"Eq{b % 3}_{pr}")
            nc.scalar.activation(Eq[:], pqs[pr][:], AF.Exp)
            Eqs[pr] = Eq

        for step in range(PAIRS + 1):
            if step < PAIRS:
                q1(step)
            if step >= 1:
                q2(step - 1)

    kvB_all = {j: kvBall[:, j, :] for j in range(B * PAIRS)}

    # ================= stage B =================
    def attn_pass1(b, attnT, rdens):
        Eqs = Eq_all[b]

        def p1(pr):
            kvB = kvB_all[b * PAIRS + pr]
            pnum = PP.tile([128, NT], F32, tag="pbig")
            nc.tensor.matmul(pnum[:], kvB[:, 0:128], Eqs[pr][:], start=True,
                             stop=True)
            pden = PD.tile([2, NT], F32, tag="pdd")
            nc.tensor.matmul(pden[:], kvB[:, 128:130], Eqs[pr][:], start=True,
                             stop=True)
            nc.scalar.activation(attnT[:, pr, :], pnum[:], AF.Copy)
            rdf = psm.tile([2, NT], F32, tag="lnden")
            nc.vector.reciprocal_approx_fast(rdf[:], pden[:])
            rdb = psm.tile([2, NT], BF16, tag="rden", bufs=2)
            nc.scalar.activation(rdb[:], rdf[:], AF.Copy)
            rdens[pr] = rdb

        for pr in range(PAIRS):
            p1(pr)

    def attn_pass2(b, attnT, rdens):
        def p2(pr):
            prdB = PP.tile([128, NT], F32, tag="pbig")
            nc.tensor.matmul(prdB[:], sel2b_bf[:], rdens[pr][:],
                             start=True, stop=True)
            nc.vector.tensor_tensor(attnT[:, pr, :], attnT[:, pr, :], prdB[:],
                                    op=ALU.mult)

        for pr in range(PAIRS):
            p2(pr)

    def stats_tiles():
        pm = PS.tile([33, NT], F32, tag="pstat")
        return pm

    def stats_step(pm, res_slice, first, last):
        sqt = psm.tile([128, NT], BF16, tag="lnsq")
        nc.vector.tensor_tensor(sqt[:], res_slice, res_slice, op=ALU.mult)
        nc.tensor.matmul(pm[0:1, :], mean1_bf[:], res_slice,
                         start=first, stop=last, skip_group_check=True)
        nc.tensor.matmul(pm[32:33, :], mean1_bf[:], sqt[:],
                         start=first, stop=last, skip_group_check=True)

    def ln_chain(pm):
        I32 = mybir.dt.int32
        mu_sb = psm.tile([1, NT], BF16, tag="lnmu", bufs=2)
        nc.scalar.activation(mu_sb[:], pm[0:1, :], AF.Copy)
        mu2 = psm.tile([1, NT], F32, tag="lnrow")
        nc.scalar.square(mu2[:], mu_sb[:])
        vare = psm.tile([1, NT], F32, tag="lnvare", bufs=1)
        nc.vector.scalar_tensor_tensor(vare[:], pm[32:33, :], float(EPS_LN),
                                       mu2[:], op0=ALU.add, op1=ALU.subtract)
        # rstd via bit-magic Newton on the vector engine (no act-table cost)
        sh = psm.tile([1, NT], I32, tag="lnrow")
        nc.vector.tensor_scalar(sh[:], vare[:].bitcast(I32), 1, None,
                                op0=ALU.arith_shift_right)
        y0 = psm.tile([1, NT], I32, tag="lnya", bufs=1)
        nc.vector.tensor_tensor(y0[:], magicrow[:], sh[:], op=ALU.subtract)
        y = y0[:].bitcast(F32)
        t1 = psm.tile([1, NT], F32, tag="lnrow")
        nc.vector.tensor_tensor(t1[:], y, y, op=ALU.mult)
        t2 = psm.tile([1, NT], F32, tag="lnrow")
        nc.vector.tensor_tensor(t2[:], t1[:], vare[:], op=ALU.mult)
        t3 = psm.tile([1, NT], F32, tag="lnrow")
        nc.scalar.activation(t3[:], t2[:], AF.Identity, bias=c15[:], scale=-0.5)
        rstd = psm.tile([1, NT], BF16, tag="lnrstd", bufs=2)
        nc.vector.tensor_tensor(rstd[:], y, t3[:], op=ALU.mult)
        nmr = psm.tile([1, NT], BF16, tag="lnnmr", bufs=2)
        nc.vector.scalar_tensor_tensor(nmr[:], mu_sb[:], -1.0, rstd[:],
                                       op0=ALU.mult, op1=ALU.mult)
        return rstd, nmr

    def ln_ab(rstd, nmr):
        pa = PD.tile([128, NT], F32, tag="pdd")
        nc.tensor.matmul(pa[:], ones1x128_bf[:], rstd[:], start=True, stop=True)
        a_bf = psm.tile([128, NT], BF16, tag="lnA", bufs=2)
        nc.scalar.activation(a_bf[:], pa[:], AF.Copy)
        pb = PD.tile([128, NT], F32, tag="pdd")
        nc.tensor.matmul(pb[:], ones1x128_bf[:], nmr[:], start=True, stop=True)
        b_bf = psm.tile([128, NT], BF16, tag="lnB", bufs=2)
        nc.scalar.activation(b_bf[:], pb[:], AF.Copy)
        return a_bf, b_bf

    def ln_apply(a_bf, b_bf, res, gc, bc, odt, dma_out=None):
        o = None
        if odt is not None:
            o = pbig.tile([128, KT_D, NT], odt, tag="bf8")
        for kt in range(KT_D):
            t1 = psm.tile([128, NT], BF16, tag="lnt1")
            nc.vector.tensor_tensor(t1[:], res[:, kt, :], a_bf[:], op=ALU.mult)
            if trivial_ln:
                if o is not None:
                    nc.vector.tensor_tensor(o[:, kt, :], t1[:], b_bf[:],
                                            op=ALU.add)
                else:
                    ot = psm.tile([128, NT], F32, tag="lnot")
                    nc.vector.tensor_tensor(ot[:], t1[:], b_bf[:], op=ALU.add)
                    nc.sync.dma_start(
                        dma_out[0][dma_out[1], kt * 128:(kt + 1) * 128, :],
                        ot[:])
            else:
                nrm = psm.tile([128, NT], BF16, tag="lnnrm")
                nc.vector.tensor_tensor(nrm[:], t1[:], b_bf[:], op=ALU.add)
                if o is not None:
                    nc.vector.tensor_scalar(o[:, kt, :], nrm[:],
                                            gc[:, kt:kt + 1], bc[:, kt:kt + 1],
                                            op0=ALU.mult, op1=ALU.add)
                else:
                    ot = psm.tile([128, NT], F32, tag="lnot")
                    nc.vector.tensor_scalar(ot[:], nrm[:], gc[:, kt:kt + 1],
                                            bc[:, kt:kt + 1],
                                            op0=ALU.mult, op1=ALU.add)
                    nc.sync.dma_start(
                        dma_out[0][dma_out[1], kt * 128:(kt + 1) * 128, :],
                        ot[:])
        return o

    def wo_res1(b, attnT):
        res1 = pbig.tile([128, KT_D, NT], BF16, tag="resX", bufs=2)
        pm = stats_tiles()
        for mt in range(KT_D):
            womt = pstream.tile([128, KT_D, 128], BF16, tag="wmt")
            nc.sync.dma_start(womt[:], wos[mt])
            po = PP.tile([128, NT], F32, tag="pbig")
            for kt in range(KT_D):
                nc.tensor.matmul(po[:], womt[:, kt, :], attnT[:, kt, :],
                                 start=kt == 0, stop=kt == KT_D - 1)
            xf = psm.tile([128, NT], BF16, tag="xf")
            nc.sync.dma_start(xf[:], x_bf[b, mt * 128:(mt + 1) * 128, :])
            nc.vector.tensor_tensor(res1[:, mt, :], xf[:], po[:], op=ALU.add)
        # stats as one block AFTER the Wo stream: by the time the PE FIFO
        # reaches them the res1 adds (queued behind the LN applies on the
        # vector engine) have drained, so the stats matmuls never stall.
        for mt in range(KT_D):
            stats_step(pm, res1[:, mt, :], mt == 0, mt == KT_D - 1)
        return res1, pm

    def ffn(b, out1, defer_n=1):
        hsb = pbig.tile([128, MT_FF, NT], BF16, tag="big32")
        res2 = pbig.tile([128, KT_D, NT], BF16, tag="resX", bufs=2)
        pm = stats_tiles()

        g0 = {}

        def ffn2_head(mt, kt_end):
            w2a = pw2s.tile([128, MT_FF // 2, 128], BF16, tag="w2mt")
            nc.sync.dma_start(w2a[:], w2s[mt, :, 0:MT_FF // 2])
            w2b = pw2s.tile([128, MT_FF // 2, 128], BF16, tag="w2mt")
            nc.sync.dma_start(w2b[:], w2s[mt, :, MT_FF // 2:MT_FF])
            pf = PP.tile([128, NT], F32, tag="pbig")
            for kt in range(kt_end):
                w2h = w2a if kt < MT_FF // 2 else w2b
                nc.tensor.matmul(pf[:], w2h[:, kt % (MT_FF // 2), :],
                                 hsb[:, kt, :],
                                 start=kt == 0, stop=False,
                                 skip_group_check=True)
            return pf, w2a, w2b

        def ffn2_tail(mt, pf, w2a, w2b, kt_start):
            for kt in range(kt_start, MT_FF):
                w2h = w2a if kt < MT_FF // 2 else w2b
                nc.tensor.matmul(pf[:], w2h[:, kt % (MT_FF // 2), :],
                                 hsb[:, kt, :],
                                 start=kt == 0, stop=kt == MT_FF - 1,
                                 skip_group_check=True)
            nc.vector.scalar_tensor_tensor(res2[:, mt, :], pf[:],
                                           cAPs["b2adjc"][:, mt:mt + 1],
                                           out1[:, mt, :], op0=ALU.add,
                                           op1=ALU.add)

        def ffn2_group(mt):
            pf, w2a, w2b = ffn2_head(mt, 0)
            ffn2_tail(mt, pf, w2a, w2b, 0)

        for mt in range(MT_FF):
            w1mt = pstream.tile([128, KT_D, 128], BF16, tag="wmt")
            nc.sync.dma_start(w1mt[:], w1s[mt])
            # 6-deep pz rotation: 4 PP banks + the 2 PD banks.
            if mt % 3 != 2:
                pz = PP.tile([128, NT], F32, tag="pbig", name="pz")
            else:
                pz = PD.tile([128, NT], F32, tag="pdd", name="pz")
            for kt in range(KT_D):
                nc.tensor.matmul(pz[:], w1mt[:, kt, :], out1[:, kt, :],
                                 start=kt == 0, stop=kt == KT_D - 1)
            # elu(z')+1 = max(z'+1, min(exp(z'), 1)): one scalar Exp, with the
            # min on the vector engine in 16-bit 2x mode.
            eraw = pmt.tile([128, NT], BF16, tag="t512bf")
            nc.scalar.activation(eraw[:], pz[:], AF.Exp,
                                 bias=cAPs["b1c"][:, mt:mt + 1])
            emin = pmt.tile([128, NT], BF16, tag="t512bf")
            nc.vector.tensor_scalar(emin[:], eraw[:], 1.0, None, op0=ALU.min)
            nc.vector.scalar_tensor_tensor(hsb[:, mt, :], pz[:],
                                           cAPs["b1p1c"][:, mt:mt + 1], emin[:],
                                           op0=ALU.add, op1=ALU.max)
            # FFN2's first accumulation group starts before FFN1's last
            # tiles (only the already-written hsb kts), so the seam never
            # waits on the trailing hsb chain.
            if mt == MT_FF - 3:
                g0['h'] = ffn2_head(0, MT_FF - 2)

        ffn2_tail(0, *g0['h'], MT_FF - 2)
        n_inline = KT_D - defer_n
        for mt in range(1, n_inline):
            ffn2_group(mt)
        # stats as one block AFTER the inline groups: each group's stats no
        # longer stall the PE FIFO on that group's trailing res2 vector op.
        for mt in range(n_inline):
            stats_step(pm, res2[:, mt, :], mt == 0,
                       defer_n == 0 and mt == KT_D - 1)
        if defer_n == 0:
            return res2, pm, None

        # the last FFN2 group(s) are deferred into the next batch's window,
        # where their (ready) matmuls cover the LN chain latencies.
        def defer():
            for mt in range(n_inline, KT_D):
                ffn2_group(mt)
            for mt in range(n_inline, KT_D):
                stats_step(pm, res2[:, mt, :], False, mt == KT_D - 1)
        return res2, pm, defer

    # pre-loop: batch 0/1 kv summaries landed during stage A, so attention
    # for batch 0 interleaves with the q-side GEMMs -- the PE stream stays
    # dense while the remaining AllReduces complete in the background.
    kv_unpack(H1, 16, 8)
    emit_qside(0)
    attnT0 = pbig.tile([128, KT_D, NT], BF16, tag="attnT")
    rd0 = {}
    attn_pass1(0, attnT0, rd0)
    emit_qside(1)
    attn_pass2(0, attnT0, rd0)
    emit_qside(2)
    res1, pm1t = wo_res1(0, attnT0)

    gN = (lambda k: cAPs[k]) if not trivial_ln else (lambda k: None)
    res2p = pm2p = defer2 = None
    for b in range(B):
        # LN2 of the previous batch is handled HERE so this batch's (ready)
        # attention/Wo matmuls cover both LN chains' serial latency.
        if defer2 is not None:
            defer2()
        rn2 = ln_chain(pm2p) if pm2p is not None else None
        rstd1, nmr1 = ln_chain(pm1t)
        attnT = rdn = None
        if b + 1 < B:
            attnT = pbig.tile([128, KT_D, NT], BF16, tag="attnT")
            rdn = {}
            attn_pass1(b + 1, attnT, rdn)
        if rn2 is not None:
            a2, b2t = ln_ab(*rn2)
        a1, b1t = ln_ab(rstd1, nmr1)
        if attnT is not None:
            attn_pass2(b + 1, attnT, rdn)
        out1 = ln_apply(a1, b1t, res1, gN("g1c"), gN("be1c"), BF16)
        if rn2 is not None:
            ln_apply(a2, b2t, res2p, gN("g2c"), gN("be2c"), None,
                     dma_out=(out, b - 1))
        if b == 0:
            emit_qside(3)
            kv_unpack(H1 + SEG, 24, 8)
        if b + 1 < B:
            res1, pm1t = wo_res1(b + 1, attnT)
        # the no-filler final iteration gets TWO deferred groups of PE cover
        res2p, pm2p, defer2 = ffn(b, out1,
                                  defer_n=0 if b == B - 1 else
                                  (2 if b == B - 2 else 1))

    rn2 = ln_chain(pm2p)
    a2, b2t = ln_ab(*rn2)
    ln_apply(a2, b2t, res2p, gN("g2c"), gN("be2c"), None, dma_out=(out, B - 1))

    ctx.close()


_CACHE = {}


def _build(trivial_ln):
    import concourse.tile as tile
    from concourse import bacc
    nc = bacc.Bacc("TRN2", target_bir_lowering=False, debug=False, num_devices=NC)
    with tile.TileContext(nc) as tc:
        _emit(nc, tc, trivial_ln)
    nc.compile()
    return nc


def _host_inputs(x, Wq, Wk, Wv, Wo, proj, W1, b1, W2, b2,
                 ln1_g, ln1_b, ln2_g, ln2_b, trivial_ln):
    bf = ml_dtypes.bfloat16
    f32 = np.float32
    d = {}

    def chunked(w):  # [D, X] -> [X/128 mt, 128 p, D/128 kt, 128]
        Dk, X = w.shape
        r = w.reshape(Dk // 128, 128, X // 128, 128)
        return np.ascontiguousarray(r.transpose(2, 1, 0, 3)).astype(bf)

    # fused q-side weights: Wq' = einsum(Wq, proj*dn) -> [D, H*M]
    Wqp = np.einsum('dhk,mk->dhm', Wq.astype(np.float64),
                    (proj.astype(np.float64) * DN)).reshape(D, H * M)
    d["wqps"] = chunked(Wqp.astype(f32))
    d["wks"] = chunked(Wk.reshape(D, D))
    d["wv"] = np.ascontiguousarray(Wv.reshape(D, D)).astype(bf)
    d["wos"] = chunked(Wo.reshape(D, D))
    d["w1s"] = chunked(W1)
    d["w2s"] = chunked(W2)

    projT_s = (proj * DN).T.astype(f32)
    pbd = np.zeros((128, 128), f32)
    pbd[0:64, 0:64] = projT_s
    pbd[64:128, 64:128] = projT_s
    d["projbd"] = pbd.astype(bf)
    nsF = np.zeros((2, 128), f32)
    nsF[0, 0:64] = -DN2H
    nsF[1, 64:128] = -DN2H
    d["negselF"] = nsF.astype(bf)
    s2 = np.zeros((128, 2), f32)
    s2[0:64, 0] = 1.0
    s2[64:128, 1] = 1.0
    d["sel2"] = s2.astype(bf)
    s2b = np.zeros((2, 128), f32)
    s2b[0, 0:64] = 1.0
    s2b[1, 64:128] = 1.0
    d["sel2b"] = s2b
    d["ones1x128"] = np.ones((1, 128), f32)
    d["mean1"] = np.full((128, 1), 1.0 / D, f32)
    hm2 = np.zeros((128, 2), f32)
    hm2[0:64, 0] = 1.0
    hm2[64:128, 1] = 1.0
    d["headmask2"] = hm2

    d["b1c"] = np.ascontiguousarray(b1.reshape(MT_FF, 128).T).astype(f32)
    d["b1p1c"] = np.ascontiguousarray((b1 + 1.0).reshape(MT_FF, 128).T).astype(f32)
    b2adj = b2.astype(np.float64) - W2.astype(np.float64).sum(axis=0)
    d["b2adjc"] = np.ascontiguousarray(b2adj.reshape(KT_D, 128).T).astype(f32)
    if not trivial_ln:
        d["g1c"] = np.ascontiguousarray(ln1_g.reshape(KT_D, 128).T).astype(f32)
        d["be1c"] = np.ascontiguousarray(ln1_b.reshape(KT_D, 128).T).astype(f32)
        d["g2c"] = np.ascontiguousarray(ln2_g.reshape(KT_D, 128).T).astype(f32)
        d["be2c"] = np.ascontiguousarray(ln2_b.reshape(KT_D, 128).T).astype(f32)
    return d


def kernel(x, Wq, Wk, Wv, Wo, proj, W1, b1, W2, b2, ln1_g, ln1_b, ln2_g, ln2_b):
    from concourse import bass_utils

    x = np.asarray(x, np.float32)
    ln1_g = np.asarray(ln1_g); ln1_b = np.asarray(ln1_b)
    ln2_g = np.asarray(ln2_g); ln2_b = np.asarray(ln2_b)
    trivial_ln = (np.allclose(ln1_g, 1.0) and np.allclose(ln2_g, 1.0)
                  and np.allclose(ln1_b, 0.0) and np.allclose(ln2_b, 0.0))
    shared = _host_inputs(x, np.asarray(Wq), np.asarray(Wk), np.asarray(Wv),
                          np.asarray(Wo), np.asarray(proj), np.asarray(W1),
                          np.asarray(b1), np.asarray(W2), np.asarray(b2),
                          ln1_g, ln1_b, ln2_g, ln2_b, trivial_ln)

    key = ("nc", trivial_ln)
    if key not in _CACHE:
        _CACHE[key] = _build(trivial_ln)
    nc = _CACHE[key]

    in_maps = []
    for c in range(NC):
        xs = x[:, c * NT:(c + 1) * NT, :]
        xT = np.ascontiguousarray(xs.transpose(0, 2, 1))
        m = dict(shared)
        m["x_bf"] = xT.astype(ml_dtypes.bfloat16)
        in_maps.append(m)

    trace = bool(int(os.environ.get("KERNEL_TRACE", "0")))
    res = bass_utils.run_bass_kernel_spmd(nc, in_maps, core_ids=list(range(NC)),
                                          trace=trace)
    if trace and res.exec_time_ns is not None:
        print(f"HW exec time: {res.exec_time_ns} ns")
        if res.instructions_and_trace is not None:
            print("trace:", res.instructions_and_trace[1])

    outp = np.empty((B, N, D), np.float32)
    for c in range(NC):
        oT = res.results[c]["out"]
        outp[:, c * NT:(c + 1) * NT, :] = oT.transpose(0, 2, 1)
    return outp
